# revision 6
# baseline (speedup 1.0000x reference)
"""Depthwise cross-correlation (per-sample dynamic kernel) on 8 Trainium2 cores.

reference: out[b,i,j,c] = sum_{di,dj} search[b,i+di,j+dj,c] * template[b,di,dj,c]
  search [64,31,31,256] f32, template [64,7,7,256] f32 -> out [64,25,25,256] f32

Strategy (pure data parallel, 8 samples/core, no collectives):
- Host marshals a channel-major bf16 blob per sample: per channel-half
  [128 part, 2 x 1024 (search 961 | pad)] plus an f32 t-column tensor
  [128, 2 x 64] (49 taps). bf16 halves DMA vs f32 and unlocks DVE 2x/4x
  modes; absmax rel err stays ~3e-3, well under the 2e-2 gate.
- The 49 taps are split across all four compute engines:
  * PE: diag(t_k) @ shift_k(S) accumulated in PSUM (bf16 matmuls, 25-wide
    rows, 13+12 row split across two PSUM banks). Diag tiles are built
    ON-CHIP (identity x t_col per-partition scale) instead of DMA'd -
    that DMA was the baseline's main bottleneck (4.5MB/sample).
  * DVE: taps as tensor_scalar mul (bf16 4x mode) + tensor_tensor add
    (2x mode) into an SBUF accumulator chain.
  * Pool (gpsimd): taps as fused scalar_tensor_tensor accumulate.
  * ACT: builds most of the diag tiles (activation Copy with per-partition
    scale); optionally a few mul-only taps absorbed by PE id-matmuls.
- PE "identity matmuls" absorb the Pool accumulator (and any ACT tap
  tmps) into PSUM for free; DVE then merges PSUM + its own accumulator
  into the bf16 output staging tile (the only PSUM reads).
- A post-pass splits multi-wait instructions (walrus allows one sync-wait
  per instruction) into single-wait NoOp carriers.
"""
import sys

sys.path.insert(0, "/opt/trn_rl_repo")

import numpy as np
import concourse.bass as bass
import concourse.mybir as mybir
import concourse.tile as tile
from concourse.bass_utils import run_bass_kernel_spmd

B = 64
X, K, OUT = 31, 7, 25
CH = 256
C = 128                      # channels per half (partition dim)
N_CORES = 8
BPC = B // N_CORES           # samples per core
SECT = 1024                  # per-half search section (961 + pad)
TSECT = 64                   # per-half t-column section (49 + pad)
R0, R1 = 13, 12              # output row split across two PSUM banks

# tap allocation: 49 taps -> PE / DVE / Pool (no ACT taps; ACT builds diags)
N_PE = 33
N_DVE = 13
N_POOL = 3
N_ACT = 49 - N_PE - N_DVE - N_POOL
# diag builders for the N_PE taps: ACT gets DIAG_ACT of them, Pool gets
# DIAG_POOL, DVE builds the rest
DIAG_ACT = 27
DIAG_POOL = 6

_CACHE = {}


def _rows(blob_s, h, di, dj, r_base, nrows):
    """[C, nrows, 25] bf16 view of search rows r_base+di..+nrows, col dj.."""
    off = h * SECT + (r_base + di) * X + dj
    return blob_s[:, off : off + X * nrows].rearrange(
        "c (r j) -> c r j", j=X)[:, :, 0:OUT]


def _corr_half(nc, sb, ps, ident, blob_s, tcol_s, h, out_view):
    f32 = mybir.dt.float32
    bf16 = mybir.dt.bfloat16
    taps = [(k // K, k % K) for k in range(K * K)]
    pe_taps = taps[:N_PE]
    dve_taps = taps[N_PE:N_PE + N_DVE]
    pool_taps = taps[N_PE + N_DVE:N_PE + N_DVE + N_POOL]
    act_taps = taps[N_PE + N_DVE + N_POOL:]

    def t_ap(k):
        return tcol_s[:, h * TSECT + k : h * TSECT + k + 1]

    # --- on-chip diag tiles for the PE taps (identity * t_col) ---
    diag = sb.tile([C, N_PE, C], bf16, tag="diag")
    for i in range(N_PE):
        k = i
        if i < DIAG_ACT:
            nc.scalar.mul(diag[:, i, :], ident[:, :], t_ap(k))
        elif i < DIAG_ACT + DIAG_POOL:
            nc.gpsimd.tensor_scalar_mul(diag[:, i, :], ident[:, :], t_ap(k))
        else:
            nc.vector.tensor_scalar_mul(diag[:, i, :], ident[:, :], t_ap(k))

    # --- PE: diag matmuls accumulating over the PE taps ---
    pa = ps.tile([C, R0, OUT], f32, tag="pa")
    pb = ps.tile([C, R1, OUT], f32, tag="pb")
    for (pt, r_base, nrows) in [(pa, 0, R0), (pb, R0, R1)]:
        for i, (di, dj) in enumerate(pe_taps):
            nc.tensor.matmul(pt[:, :, :], diag[:, i, :],
                             _rows(blob_s, h, di, dj, r_base, nrows),
                             start=(i == 0), stop=(i == len(pe_taps) - 1),
                             skip_group_check=True)

    # --- DVE: tensor_scalar mul + tensor_tensor add chain (bf16) ---
    acc_d = sb.tile([C, OUT, OUT], bf16, tag="acc_d")
    for i, (di, dj) in enumerate(dve_taps):
        k = N_PE + i
        rows = _rows(blob_s, h, di, dj, 0, OUT)
        if i == 0:
            nc.vector.tensor_scalar_mul(acc_d[:, :, :], rows, t_ap(k))
        else:
            tmp = sb.tile([C, OUT, OUT], bf16, tag="tmp_d")
            nc.vector.tensor_scalar_mul(tmp[:, :, :], rows, t_ap(k))
            nc.vector.tensor_add(out=acc_d[:, :, :], in0=acc_d[:, :, :],
                                 in1=tmp[:, :, :])

    # --- Pool: tensor_scalar mul + tensor_tensor add chain (bf16) ---
    # (scalar_tensor_tensor doesn't pass the walrus ISA check on Pool)
    acc_p = sb.tile([C, OUT, OUT], bf16, tag="acc_p")
    for i, (di, dj) in enumerate(pool_taps):
        k = N_PE + N_DVE + i
        rows = _rows(blob_s, h, di, dj, 0, OUT)
        if i == 0:
            nc.gpsimd.tensor_scalar_mul(acc_p[:, :, :], rows, t_ap(k))
        else:
            tmp = sb.tile([C, OUT, OUT], bf16, tag="tmp_p")
            nc.gpsimd.tensor_scalar_mul(tmp[:, :, :], rows, t_ap(k))
            nc.gpsimd.tensor_add(out=acc_p[:, :, :], in0=acc_p[:, :, :],
                                 in1=tmp[:, :, :])

    # --- Pool: merge its accumulator into DVE's (keeps PE free) ---
    acc_m = sb.tile([C, OUT, OUT], bf16, tag="acc_m")
    nc.gpsimd.tensor_add(out=acc_m[:, :, :], in0=acc_d[:, :, :],
                         in1=acc_p[:, :, :])

    # --- DVE: merge PSUM + merged accumulator -> bf16 output staging ---
    nc.vector.tensor_add(out=out_view[:, 0:R0, :], in0=pa[:, :, :],
                         in1=acc_m[:, 0:R0, :])
    nc.vector.tensor_add(out=out_view[:, R0:OUT, :], in0=pb[:, :, :],
                         in1=acc_m[:, R0:OUT, :])


def _split_excess_waits(nc):
    """Walrus codegen allows a single sync-wait command per instruction.
    Move extra waits onto inserted same-engine NoOps; firing a monotone
    wait earlier on the same queue is always safe."""
    for fn in nc.m.functions:
        for bb in fn.blocks:
            out = []
            for inst in bb.instructions:
                si = inst.sync_info
                if si is not None and len(si.on_wait) > 1:
                    waits = list(si.on_wait)
                    for w in waits[:-1]:
                        nop = mybir.InstNoOp(
                            name=nc.get_next_instruction_name(), ins=[], outs=[])
                        nop.engine = inst.engine
                        nop.sync_info = mybir.SyncInfo(on_wait=[w], on_update=[])
                        out.append(nop)
                    si.on_wait = [waits[-1]]
                out.append(inst)
            bb.instructions = out


def _build_nc(reps=1):
    assert N_PE + N_DVE + N_POOL == K * K, "all 49 taps must be assigned"
    bf16 = mybir.dt.bfloat16
    nc = bass.Bass("TRN2", debug=False)
    b_in = nc.dram_tensor("blob", [BPC, C, 2 * SECT], bf16,
                          kind="ExternalInput").ap()
    t_in = nc.dram_tensor("tcol", [BPC, C, 2 * TSECT], mybir.dt.float32,
                          kind="ExternalInput").ap()
    i_in = nc.dram_tensor("ident", [C, C], bf16, kind="ExternalInput").ap()
    o_out = nc.dram_tensor("o", [BPC, C, 2, OUT, OUT], bf16,
                           kind="ExternalOutput").ap()
    with tile.TileContext(nc) as tc:
        with tc.tile_pool(name="const", bufs=1) as const, \
             tc.tile_pool(name="sb", bufs=2) as sb, \
             tc.tile_pool(name="work", bufs=3) as work, \
             tc.tile_pool(name="ps", bufs=4, space="PSUM") as ps:
            ident = const.tile([C, C], bf16, tag="ident")
            nc.sync.dma_start(out=ident[:], in_=i_in)
            for _ in range(reps):
                for s in range(BPC):
                    blob_s = sb.tile([C, 2 * SECT], bf16, tag="blob")
                    nc.sync.dma_start(out=blob_s[:], in_=b_in[s])
                    tcol_s = sb.tile([C, 2 * TSECT], mybir.dt.float32,
                                     tag="tcol")
                    nc.sync.dma_start(out=tcol_s[:], in_=t_in[s])
                    out_sb = work.tile([C, 2, OUT, OUT], bf16, tag="out_sb")
                    for h in range(2):
                        _corr_half(nc, work, ps, ident, blob_s, tcol_s, h,
                                   out_sb[:, h])
                    nc.sync.dma_start(out=o_out[s], in_=out_sb[:])
    _split_excess_waits(nc)
    return nc


def _marshal(search, template):
    """-> blob [B, C, 2*SECT] bf16, tcol [B, C, 2*TSECT] f32, ident bf16."""
    import ml_dtypes
    search = np.ascontiguousarray(search, dtype=np.float32)
    template = np.ascontiguousarray(template, dtype=np.float32)
    s_cm = search.reshape(B, X * X, 2, C).transpose(0, 2, 3, 1)   # [B,2,C,961]
    t_cm = template.reshape(B, K * K, 2, C).transpose(0, 2, 3, 1)  # [B,2,C,49]
    blob = np.zeros((B, 2, C, SECT), np.float32)
    blob[:, :, :, :X * X] = s_cm
    blob = blob.transpose(0, 2, 1, 3).reshape(B, C, 2 * SECT)
    tcol = np.zeros((B, 2, C, TSECT), np.float32)
    tcol[:, :, :, :K * K] = t_cm
    tcol = np.ascontiguousarray(
        tcol.transpose(0, 2, 1, 3).reshape(B, C, 2 * TSECT))
    ident = np.eye(C, dtype=ml_dtypes.bfloat16)
    return blob.astype(ml_dtypes.bfloat16), tcol, ident


def _unmarshal(results):
    o = np.stack([np.asarray(results[core]["o"]).astype(np.float32)
                  for core in range(N_CORES)])
    # [cores, BPC, C, 2, OUT, OUT] -> [B, OUT, OUT, 2, C] -> [B, OUT, OUT, CH]
    o = o.reshape(B, C, 2, OUT, OUT).transpose(0, 3, 4, 2, 1)
    return np.ascontiguousarray(o.reshape(B, OUT, OUT, CH))


def kernel(search, template):
    if "nc" not in _CACHE:
        _CACHE["nc"] = _build_nc()
    nc = _CACHE["nc"]
    blob, tcol, ident = _marshal(search, template)
    blob = blob.reshape(N_CORES, BPC, C, 2 * SECT)
    tcol = tcol.reshape(N_CORES, BPC, C, 2 * TSECT)
    in_maps = [{"blob": blob[core], "tcol": tcol[core], "ident": ident}
               for core in range(N_CORES)]
    res = run_bass_kernel_spmd(nc, in_maps, core_ids=list(range(N_CORES)))
    return _unmarshal(res.results)


# revision 7
# speedup vs baseline: 3.9897x; 3.9897x over previous
"""Depthwise cross-correlation (per-sample dynamic kernel) on 8 Trainium2 cores.

reference: out[b,i,j,c] = sum_{di,dj} search[b,i+di,j+dj,c] * template[b,di,dj,c]
  search [64,31,31,256] f32, template [64,7,7,256] f32 -> out [64,25,25,256] f32

Strategy (pure data parallel, 8 samples/core, no collectives):
- Host marshals a channel-major bf16 blob per sample: per channel-half
  [128 part, 2 x 1024 (search 961 | pad)] plus an f32 t-column tensor
  [128, 2 x 64] (49 taps). bf16 halves DMA vs f32 and unlocks DVE 2x/4x
  modes; absmax rel err stays ~3e-3, well under the 2e-2 gate.
- The 49 taps are split across all four compute engines:
  * PE: diag(t_k) @ shift_k(S) accumulated in PSUM (bf16 matmuls, 25-wide
    rows, 13+12 row split across two PSUM banks). Diag tiles are built
    ON-CHIP (identity x t_col per-partition scale) instead of DMA'd -
    that DMA was the baseline's main bottleneck (4.5MB/sample).
  * DVE: taps as tensor_scalar mul (bf16 4x mode) + tensor_tensor add
    (2x mode) into an SBUF accumulator chain.
  * Pool (gpsimd): taps as fused scalar_tensor_tensor accumulate.
  * ACT: builds most of the diag tiles (activation Copy with per-partition
    scale); optionally a few mul-only taps absorbed by PE id-matmuls.
- PE "identity matmuls" absorb the Pool accumulator (and any ACT tap
  tmps) into PSUM for free; DVE then merges PSUM + its own accumulator
  into the bf16 output staging tile (the only PSUM reads).
- A post-pass splits multi-wait instructions (walrus allows one sync-wait
  per instruction) into single-wait NoOp carriers.
"""
import sys

sys.path.insert(0, "/opt/trn_rl_repo")

import numpy as np
import concourse.bass as bass
import concourse.mybir as mybir
import concourse.tile as tile
from concourse.bass_utils import run_bass_kernel_spmd

B = 64
X, K, OUT = 31, 7, 25
CH = 256
C = 128                      # channels per half (partition dim)
N_CORES = 8
BPC = B // N_CORES           # samples per core
SECT = 1024                  # per-half search section (961 + pad)
TSECT = 64                   # per-half t-column section (49 + pad)
R0, R1 = 13, 12              # output row split across two PSUM banks

# tap allocation: 49 taps -> PE / DVE / Pool (no ACT taps; ACT builds diags)
N_PE = 33
N_DVE = 13
N_POOL = 3
N_ACT = 49 - N_PE - N_DVE - N_POOL
# diag builders for the N_PE taps: ACT gets DIAG_ACT of them, Pool gets
# DIAG_POOL, DVE builds the rest
DIAG_ACT = 27
DIAG_POOL = 6

_CACHE = {}


def _rows(blob_s, h, di, dj, r_base, nrows):
    """[C, nrows, 25] bf16 view of search rows r_base+di..+nrows, col dj.."""
    off = h * SECT + (r_base + di) * X + dj
    return blob_s[:, off : off + X * nrows].rearrange(
        "c (r j) -> c r j", j=X)[:, :, 0:OUT]


def _corr_half(nc, sb, ps, ident, blob_s, tcol_s, h, out_view):
    f32 = mybir.dt.float32
    bf16 = mybir.dt.bfloat16
    taps = [(k // K, k % K) for k in range(K * K)]
    pe_taps = taps[:N_PE]
    dve_taps = taps[N_PE:N_PE + N_DVE]
    pool_taps = taps[N_PE + N_DVE:N_PE + N_DVE + N_POOL]
    act_taps = taps[N_PE + N_DVE + N_POOL:]

    def t_ap(k):
        return tcol_s[:, h * TSECT + k : h * TSECT + k + 1]

    # --- on-chip diag tiles for the PE taps (identity * t_col) ---
    diag = sb.tile([C, N_PE, C], bf16, tag="diag")
    for i in range(N_PE):
        k = i
        if i < DIAG_ACT:
            nc.scalar.mul(diag[:, i, :], ident[:, :], t_ap(k))
        elif i < DIAG_ACT + DIAG_POOL:
            nc.gpsimd.tensor_scalar_mul(diag[:, i, :], ident[:, :], t_ap(k))
        else:
            nc.vector.tensor_scalar_mul(diag[:, i, :], ident[:, :], t_ap(k))

    # --- PE: diag matmuls accumulating over the PE taps ---
    pa = ps.tile([C, R0, OUT], f32, tag="pa")
    pb = ps.tile([C, R1, OUT], f32, tag="pb")
    for (pt, r_base, nrows) in [(pa, 0, R0), (pb, R0, R1)]:
        for i, (di, dj) in enumerate(pe_taps):
            nc.tensor.matmul(pt[:, :, :], diag[:, i, :],
                             _rows(blob_s, h, di, dj, r_base, nrows),
                             start=(i == 0), stop=(i == len(pe_taps) - 1),
                             skip_group_check=True)

    # --- DVE: tensor_scalar mul + tensor_tensor add chain (bf16) ---
    acc_d = sb.tile([C, OUT, OUT], bf16, tag="acc_d")
    for i, (di, dj) in enumerate(dve_taps):
        k = N_PE + i
        rows = _rows(blob_s, h, di, dj, 0, OUT)
        if i == 0:
            nc.vector.tensor_scalar_mul(acc_d[:, :, :], rows, t_ap(k))
        else:
            tmp = sb.tile([C, OUT, OUT], bf16, tag="tmp_d")
            nc.vector.tensor_scalar_mul(tmp[:, :, :], rows, t_ap(k))
            nc.vector.tensor_add(out=acc_d[:, :, :], in0=acc_d[:, :, :],
                                 in1=tmp[:, :, :])

    # --- Pool: tensor_scalar mul + tensor_tensor add chain (bf16) ---
    # (scalar_tensor_tensor doesn't pass the walrus ISA check on Pool)
    if pool_taps:
        acc_p = sb.tile([C, OUT, OUT], bf16, tag="acc_p")
        for i, (di, dj) in enumerate(pool_taps):
            k = N_PE + N_DVE + i
            rows = _rows(blob_s, h, di, dj, 0, OUT)
            if i == 0:
                nc.gpsimd.tensor_scalar_mul(acc_p[:, :, :], rows, t_ap(k))
            else:
                tmp = sb.tile([C, OUT, OUT], bf16, tag="tmp_p")
                nc.gpsimd.tensor_scalar_mul(tmp[:, :, :], rows, t_ap(k))
                nc.gpsimd.tensor_add(out=acc_p[:, :, :], in0=acc_p[:, :, :],
                                     in1=tmp[:, :, :])
        # Pool merges its accumulator into DVE's (keeps PE free)
        acc_m = sb.tile([C, OUT, OUT], bf16, tag="acc_m")
        nc.gpsimd.tensor_add(out=acc_m[:, :, :], in0=acc_d[:, :, :],
                             in1=acc_p[:, :, :])
    else:
        acc_m = acc_d

    # --- DVE: merge PSUM + merged accumulator -> bf16 output staging ---
    nc.vector.tensor_add(out=out_view[:, 0:R0, :], in0=pa[:, :, :],
                         in1=acc_m[:, 0:R0, :])
    nc.vector.tensor_add(out=out_view[:, R0:OUT, :], in0=pb[:, :, :],
                         in1=acc_m[:, R0:OUT, :])


def _split_excess_waits(nc):
    """Walrus codegen allows a single sync-wait command per instruction.
    Move extra waits onto inserted same-engine NoOps; firing a monotone
    wait earlier on the same queue is always safe."""
    for fn in nc.m.functions:
        for bb in fn.blocks:
            out = []
            for inst in bb.instructions:
                si = inst.sync_info
                if si is not None and len(si.on_wait) > 1:
                    waits = list(si.on_wait)
                    for w in waits[:-1]:
                        nop = mybir.InstNoOp(
                            name=nc.get_next_instruction_name(), ins=[], outs=[])
                        nop.engine = inst.engine
                        nop.sync_info = mybir.SyncInfo(on_wait=[w], on_update=[])
                        out.append(nop)
                    si.on_wait = [waits[-1]]
                out.append(inst)
            bb.instructions = out


def _build_nc(reps=1):
    assert N_PE + N_DVE + N_POOL == K * K, "all 49 taps must be assigned"
    bf16 = mybir.dt.bfloat16
    nc = bass.Bass("TRN2", debug=False)
    b_in = nc.dram_tensor("blob", [BPC, C, 2 * SECT], bf16,
                          kind="ExternalInput").ap()
    t_in = nc.dram_tensor("tcol", [BPC, C, 2 * TSECT], mybir.dt.float32,
                          kind="ExternalInput").ap()
    i_in = nc.dram_tensor("ident", [C, C], bf16, kind="ExternalInput").ap()
    o_out = nc.dram_tensor("o", [BPC, C, 2, OUT, OUT], bf16,
                           kind="ExternalOutput").ap()
    with tile.TileContext(nc) as tc:
        with tc.tile_pool(name="const", bufs=1) as const, \
             tc.tile_pool(name="sb", bufs=2) as sb, \
             tc.tile_pool(name="work", bufs=3) as work, \
             tc.tile_pool(name="ps", bufs=4, space="PSUM") as ps:
            ident = const.tile([C, C], bf16, tag="ident")
            nc.sync.dma_start(out=ident[:], in_=i_in)
            for _ in range(reps):
                for s in range(BPC):
                    blob_s = sb.tile([C, 2 * SECT], bf16, tag="blob")
                    nc.sync.dma_start(out=blob_s[:], in_=b_in[s])
                    tcol_s = sb.tile([C, 2 * TSECT], mybir.dt.float32,
                                     tag="tcol")
                    nc.sync.dma_start(out=tcol_s[:], in_=t_in[s])
                    out_sb = work.tile([C, 2, OUT, OUT], bf16, tag="out_sb")
                    for h in range(2):
                        _corr_half(nc, work, ps, ident, blob_s, tcol_s, h,
                                   out_sb[:, h])
                    nc.sync.dma_start(out=o_out[s], in_=out_sb[:])
    _split_excess_waits(nc)
    return nc


def _marshal(search, template):
    """-> blob [B, C, 2*SECT] bf16, tcol [B, C, 2*TSECT] f32, ident bf16."""
    import ml_dtypes
    search = np.ascontiguousarray(search, dtype=np.float32)
    template = np.ascontiguousarray(template, dtype=np.float32)
    s_cm = search.reshape(B, X * X, 2, C).transpose(0, 2, 3, 1)   # [B,2,C,961]
    t_cm = template.reshape(B, K * K, 2, C).transpose(0, 2, 3, 1)  # [B,2,C,49]
    blob = np.zeros((B, 2, C, SECT), np.float32)
    blob[:, :, :, :X * X] = s_cm
    blob = blob.transpose(0, 2, 1, 3).reshape(B, C, 2 * SECT)
    tcol = np.zeros((B, 2, C, TSECT), np.float32)
    tcol[:, :, :, :K * K] = t_cm
    tcol = np.ascontiguousarray(
        tcol.transpose(0, 2, 1, 3).reshape(B, C, 2 * TSECT))
    ident = np.eye(C, dtype=ml_dtypes.bfloat16)
    return blob.astype(ml_dtypes.bfloat16), tcol, ident


def _unmarshal(results):
    o = np.stack([np.asarray(results[core]["o"]).astype(np.float32)
                  for core in range(N_CORES)])
    # [cores, BPC, C, 2, OUT, OUT] -> [B, OUT, OUT, 2, C] -> [B, OUT, OUT, CH]
    o = o.reshape(B, C, 2, OUT, OUT).transpose(0, 3, 4, 2, 1)
    return np.ascontiguousarray(o.reshape(B, OUT, OUT, CH))


def kernel(search, template):
    if "nc" not in _CACHE:
        _CACHE["nc"] = _build_nc()
    nc = _CACHE["nc"]
    blob, tcol, ident = _marshal(search, template)
    blob = blob.reshape(N_CORES, BPC, C, 2 * SECT)
    tcol = tcol.reshape(N_CORES, BPC, C, 2 * TSECT)
    in_maps = [{"blob": blob[core], "tcol": tcol[core], "ident": ident}
               for core in range(N_CORES)]
    res = run_bass_kernel_spmd(nc, in_maps, core_ids=list(range(N_CORES)))
    return _unmarshal(res.results)


# revision 8
# speedup vs baseline: 4.0464x; 1.0142x over previous
"""Depthwise cross-correlation (per-sample dynamic kernel) on 8 Trainium2 cores.

reference: out[b,i,j,c] = sum_{di,dj} search[b,i+di,j+dj,c] * template[b,di,dj,c]
  search [64,31,31,256] f32, template [64,7,7,256] f32 -> out [64,25,25,256] f32

Strategy (pure data parallel, 8 samples/core, no collectives):
- Host marshals a channel-major bf16 blob per sample: per channel-half
  [128 part, 2 x 1024 (search 961 | pad)] plus an f32 t-column tensor
  [128, 2 x 64] (49 taps). bf16 halves DMA vs f32 and unlocks DVE 2x/4x
  modes; absmax rel err stays ~3e-3, well under the 2e-2 gate.
- The 49 taps are split across all four compute engines:
  * PE: diag(t_k) @ shift_k(S) accumulated in PSUM (bf16 matmuls, 25-wide
    rows, 13+12 row split across two PSUM banks). Diag tiles are built
    ON-CHIP (identity x t_col per-partition scale) instead of DMA'd -
    that DMA was the baseline's main bottleneck (4.5MB/sample).
  * DVE: taps as tensor_scalar mul (bf16 4x mode) + tensor_tensor add
    (2x mode) into an SBUF accumulator chain.
  * Pool (gpsimd): taps as fused scalar_tensor_tensor accumulate.
  * ACT: builds most of the diag tiles (activation Copy with per-partition
    scale); optionally a few mul-only taps absorbed by PE id-matmuls.
- PE "identity matmuls" absorb the Pool accumulator (and any ACT tap
  tmps) into PSUM for free; DVE then merges PSUM + its own accumulator
  into the bf16 output staging tile (the only PSUM reads).
- A post-pass splits multi-wait instructions (walrus allows one sync-wait
  per instruction) into single-wait NoOp carriers.
"""
import sys

sys.path.insert(0, "/opt/trn_rl_repo")

import numpy as np
import concourse.bass as bass
import concourse.mybir as mybir
import concourse.tile as tile
from concourse.bass_utils import run_bass_kernel_spmd

B = 64
X, K, OUT = 31, 7, 25
CH = 256
C = 128                      # channels per half (partition dim)
N_CORES = 8
BPC = B // N_CORES           # samples per core
SECT = 1024                  # per-half search section (961 + pad)
TSECT = 64                   # per-half t-column section (49 + pad)
R0, R1 = 13, 12              # output row split across two PSUM banks

# tap allocation: 49 taps -> PE (DMA'd bf16 diag tiles) / DVE / ACT.
# ACT taps are mul-only; N_ACT_PE of them are absorbed into PSUM by PE
# identity matmuls, the rest by DVE tensor_tensor adds. Pool/gpsimd is
# unused: its real-HW throughput is ~10x worse than the cost model.
N_PE = 28
N_DVE = 9
N_ACT = 49 - N_PE - N_DVE
N_ACT_PE = 5
DOFF = SECT                   # diag tiles at offset 1024 within each half
HSECT = SECT + N_PE * C       # per-half blob section (search + diags)

_CACHE = {}


def _rows(blob_s, h, di, dj, r_base, nrows):
    """[C, nrows, 25] bf16 view of search rows r_base+di..+nrows, col dj.."""
    off = h * HSECT + (r_base + di) * X + dj
    return blob_s[:, off : off + X * nrows].rearrange(
        "c (r j) -> c r j", j=X)[:, :, 0:OUT]


def _corr_half(nc, sb, ps, ident, blob_s, tcol_s, h, out_view):
    f32 = mybir.dt.float32
    bf16 = mybir.dt.bfloat16
    taps = [(k // K, k % K) for k in range(K * K)]
    pe_taps = taps[:N_PE]
    dve_taps = taps[N_PE:N_PE + N_DVE]
    act_taps = taps[N_PE + N_DVE:]

    def t_ap(k):
        return tcol_s[:, h * TSECT + k : h * TSECT + k + 1]

    d_v = blob_s[:, h * HSECT + DOFF : h * HSECT + DOFF + N_PE * C].rearrange(
        "c (k m) -> c k m", k=N_PE)

    # --- PE: diag matmuls (DMA'd bf16 diag tiles) over the PE taps ---
    pa = ps.tile([C, R0, OUT], f32, tag="pa")
    pb = ps.tile([C, R1, OUT], f32, tag="pb")
    for (pt, r_base, nrows) in [(pa, 0, R0), (pb, R0, R1)]:
        for i, (di, dj) in enumerate(pe_taps):
            nc.tensor.matmul(pt[:, :, :], d_v[:, i, :],
                             _rows(blob_s, h, di, dj, r_base, nrows),
                             start=(i == 0), stop=False,
                             skip_group_check=True)

    # --- DVE: tensor_scalar mul + tensor_tensor add chain (bf16) ---
    acc_d = sb.tile([C, OUT, OUT], bf16, tag="acc_d")
    for i, (di, dj) in enumerate(dve_taps):
        k = N_PE + i
        rows = _rows(blob_s, h, di, dj, 0, OUT)
        if i == 0:
            nc.vector.tensor_scalar_mul(acc_d[:, :, :], rows, t_ap(k))
        else:
            tmp = sb.tile([C, OUT, OUT], bf16, tag="tmp_d")
            nc.vector.tensor_scalar_mul(tmp[:, :, :], rows, t_ap(k))
            nc.vector.tensor_add(out=acc_d[:, :, :], in0=acc_d[:, :, :],
                                 in1=tmp[:, :, :])

    # --- ACT: mul-only taps; first N_ACT_PE absorbed by PE id-matmuls,
    # the rest added into acc_d by DVE ---
    pe_tmps, dve_tmps = [], []
    for i, (di, dj) in enumerate(act_taps):
        k = N_PE + N_DVE + i
        rows = _rows(blob_s, h, di, dj, 0, OUT)
        tmp = sb.tile([C, OUT, OUT], bf16, tag="tmp_a")
        nc.scalar.mul(tmp[:, :, :], rows, t_ap(k))
        (pe_tmps if i < N_ACT_PE else dve_tmps).append(tmp)

    for t in dve_tmps:
        nc.vector.tensor_add(out=acc_d[:, :, :], in0=acc_d[:, :, :],
                             in1=t[:, :, :])

    for (pt, r_base, nrows) in [(pa, 0, R0), (pb, R0, R1)]:
        for j, t in enumerate(pe_tmps):
            nc.tensor.matmul(pt[:, :, :], ident[:, :],
                             t[:, r_base:r_base + nrows, :],
                             start=False, stop=(j == len(pe_tmps) - 1),
                             skip_group_check=True)

    # --- DVE: merge PSUM + accumulator -> bf16 output staging ---
    nc.vector.tensor_add(out=out_view[:, 0:R0, :], in0=pa[:, :, :],
                         in1=acc_d[:, 0:R0, :])
    nc.vector.tensor_add(out=out_view[:, R0:OUT, :], in0=pb[:, :, :],
                         in1=acc_d[:, R0:OUT, :])


def _split_excess_waits(nc):
    """Walrus codegen allows a single sync-wait command per instruction.
    Move extra waits onto inserted same-engine NoOps; firing a monotone
    wait earlier on the same queue is always safe."""
    for fn in nc.m.functions:
        for bb in fn.blocks:
            out = []
            for inst in bb.instructions:
                si = inst.sync_info
                if si is not None and len(si.on_wait) > 1:
                    waits = list(si.on_wait)
                    for w in waits[:-1]:
                        nop = mybir.InstNoOp(
                            name=nc.get_next_instruction_name(), ins=[], outs=[])
                        nop.engine = inst.engine
                        nop.sync_info = mybir.SyncInfo(on_wait=[w], on_update=[])
                        out.append(nop)
                    si.on_wait = [waits[-1]]
                out.append(inst)
            bb.instructions = out


def _build_nc(reps=1):
    assert N_PE + N_DVE + N_ACT == K * K, "all 49 taps must be assigned"
    bf16 = mybir.dt.bfloat16
    nc = bass.Bass("TRN2", debug=False)
    b_in = nc.dram_tensor("blob", [BPC, C, 2 * HSECT], bf16,
                          kind="ExternalInput").ap()
    t_in = nc.dram_tensor("tcol", [BPC, C, 2 * TSECT], mybir.dt.float32,
                          kind="ExternalInput").ap()
    i_in = nc.dram_tensor("ident", [C, C], bf16, kind="ExternalInput").ap()
    o_out = nc.dram_tensor("o", [BPC, C, 2, OUT, OUT], bf16,
                           kind="ExternalOutput").ap()
    with tile.TileContext(nc) as tc:
        with tc.tile_pool(name="const", bufs=1) as const, \
             tc.tile_pool(name="sb", bufs=2) as sb, \
             tc.tile_pool(name="work", bufs=3) as work, \
             tc.tile_pool(name="ps", bufs=4, space="PSUM") as ps:
            ident = const.tile([C, C], bf16, tag="ident")
            nc.sync.dma_start(out=ident[:], in_=i_in)
            for _ in range(reps):
                for s in range(BPC):
                    blob_s = sb.tile([C, 2 * HSECT], bf16, tag="blob")
                    nc.sync.dma_start(out=blob_s[:], in_=b_in[s])
                    tcol_s = sb.tile([C, 2 * TSECT], mybir.dt.float32,
                                     tag="tcol")
                    nc.sync.dma_start(out=tcol_s[:], in_=t_in[s])
                    out_sb = work.tile([C, 2, OUT, OUT], bf16, tag="out_sb")
                    for h in range(2):
                        _corr_half(nc, work, ps, ident, blob_s, tcol_s, h,
                                   out_sb[:, h])
                    nc.sync.dma_start(out=o_out[s], in_=out_sb[:])
    _split_excess_waits(nc)
    return nc


def _marshal(search, template):
    """-> blob [B, C, 2*SECT] bf16, tcol [B, C, 2*TSECT] f32, ident bf16."""
    import ml_dtypes
    search = np.ascontiguousarray(search, dtype=np.float32)
    template = np.ascontiguousarray(template, dtype=np.float32)
    s_cm = search.reshape(B, X * X, 2, C).transpose(0, 2, 3, 1)   # [B,2,C,961]
    t_cm = template.reshape(B, K * K, 2, C).transpose(0, 2, 3, 1)  # [B,2,C,49]
    blob = np.zeros((B, 2, C, HSECT), np.float32)
    blob[:, :, :, :X * X] = s_cm
    dd = blob[:, :, :, DOFF:].reshape(B, 2, C, N_PE, C)
    c = np.arange(C)
    dd[:, :, c, :, c] = t_cm[:, :, :, :N_PE].transpose(2, 0, 1, 3)
    blob = blob.transpose(0, 2, 1, 3).reshape(B, C, 2 * HSECT)
    tcol = np.zeros((B, 2, C, TSECT), np.float32)
    tcol[:, :, :, :K * K] = t_cm
    tcol = np.ascontiguousarray(
        tcol.transpose(0, 2, 1, 3).reshape(B, C, 2 * TSECT))
    ident = np.eye(C, dtype=ml_dtypes.bfloat16)
    return blob.astype(ml_dtypes.bfloat16), tcol, ident


def _unmarshal(results):
    o = np.stack([np.asarray(results[core]["o"]).astype(np.float32)
                  for core in range(N_CORES)])
    # [cores, BPC, C, 2, OUT, OUT] -> [B, OUT, OUT, 2, C] -> [B, OUT, OUT, CH]
    o = o.reshape(B, C, 2, OUT, OUT).transpose(0, 3, 4, 2, 1)
    return np.ascontiguousarray(o.reshape(B, OUT, OUT, CH))


def kernel(search, template):
    if "nc" not in _CACHE:
        _CACHE["nc"] = _build_nc()
    nc = _CACHE["nc"]
    blob, tcol, ident = _marshal(search, template)
    blob = blob.reshape(N_CORES, BPC, C, 2 * HSECT)
    tcol = tcol.reshape(N_CORES, BPC, C, 2 * TSECT)
    in_maps = [{"blob": blob[core], "tcol": tcol[core], "ident": ident}
               for core in range(N_CORES)]
    res = run_bass_kernel_spmd(nc, in_maps, core_ids=list(range(N_CORES)))
    return _unmarshal(res.results)
